# revision 13
# baseline (speedup 1.0000x reference)
"""SSD-style multibox loss (Huber loc + softmax conf with hard-negative
mining) on 8 Trainium2 NeuronCores, pure data-parallel over the batch.

v2: host lays out each core's 4 batch rows as flat [128, G*C] arrays
(partition 32r+q holds anchors [q*G, (q+1)*G) of batch row r, zero-copy
reshape after padding P 8732 -> 8736). Every DMA is then a clean 2D
128-partition transfer (8 descriptors per SDMA engine per dma_start,
vs 2 with the old per-row-block 32-partition DMAs -> much deeper HBM
pipelining per engine), and the two bulk queues (sync HWDGE + gpsimd
SWDGE) run pure interlock-free FIFO streams: pred chunks first (so the
softmax lse / hard-negative-mining chain hides under the label stream),
then label chunks into a 4-deep pool consumed by the DVE dot products.

Pad anchors (4 per batch row) are handled with ZERO device work by
choosing their HBM contents on the host:
  - actual/pred bbox pads = 1.0  -> pads read as "positive" (posmask=1)
    with zero Huber loss; the resulting known constants are corrected:
    on-device kcol = 3*posrep - 12, negk2 = 8760 - 6*posrep; on host
    pos -= 16 and S2 -= 16*ln(81) per core.
  - pred_labels / actual_labels pads = 0.0 -> lse_pad = ln(81) exactly
    (corrected in S2), pred0_pad = 0 (S3/S4 unaffected), dot_pad = 0
    (S1 unaffected); masked_pad = -1e30 via posmask so mining never
    selects pads.

Per core the device computes partial sums (loc, S2, S3, S4, pos, negsum,
S1 per chunk) reduced across partitions by one PE matmul; the host
combines the 8 cores' scalars:  conf = S2 - S1 + S4 - S3 + negsum, using
the one-hot identity  sum_pos dot = S1 - (S4 - S3).  Hard-negative top-k
per row via an 11-step threshold binary search (ACT sign + PE
block-replicate matmul), sum_topk = sum(v*[v>t]) + t*(k - count(v>t)).
"""

import numpy as np

import concourse.bass as bass
import concourse.bacc as bacc
import concourse.tile as tile
import concourse.mybir as mybir
from concourse.bass_utils import run_bass_kernel_spmd

F32 = mybir.dt.float32
AX = mybir.AxisListType
OP = mybir.AluOpType
AF = mybir.ActivationFunctionType

B, P, C = 32, 8732, 81
NCORES = 8
BL = B // NCORES            # batch rows per core = 4
PPR = 32                    # partitions per batch row
G = 273                     # anchors per partition (32*273 = 8736)
NPAD = PPR * G - P          # 4 pad anchors per batch row
NEG_BIG = -1.0e30
NITER = 9                   # binary-search iterations (range [0, 32))
T0 = 16.0
NF = 16                     # output partial columns

CH = 39                     # chunk size in anchor-groups (7 chunks of G)
NCH = G // CH

# column map of the [1, NF] per-core output
COL_LOC, COL_S2, COL_S3, COL_S4, COL_POS, COL_NEG = 0, 1, 2, 3, 4, 5
COL_S1 = 8                  # .. COL_S1 + NCH - 1

# DMA sub-chunk in groups: 13-group descriptors (4212B) run ~25 GB/s per
# SDMA engine vs ~18 GB/s for 39-group (12.6KB) ones.
CHD = 13

# Every transfer is split into a top-half (partitions 0-63, sync HWDGE)
# and a bottom-half (partitions 64-127, gpsimd SWDGE). The SBUF port
# swizzle maps partitions 0-63 to the even SDMA engines and 64-127 to
# the odd ones, so the two queues drive disjoint engine sets (no
# per-engine round-robin contention) and are perfectly balanced.


def build():
    nc = bacc.Bacc("TRN2", target_bir_lowering=False, debug=False)

    d_ab = nc.dram_tensor("ab", [128, G * 4], F32, kind="ExternalInput")
    d_pb = nc.dram_tensor("pb", [128, G * 4], F32, kind="ExternalInput")
    d_al = nc.dram_tensor("al", [128, G * C], F32, kind="ExternalInput")
    d_pl = nc.dram_tensor("pl", [128, G * C], F32, kind="ExternalInput")
    d_out = nc.dram_tensor("out", [1, NF], F32, kind="ExternalOutput")

    qeng = None  # set inside build body

    with tile.TileContext(nc) as tc:
        with (
            tc.tile_pool(name="const", bufs=1) as constp,
            tc.tile_pool(name="resident", bufs=1) as resp,
            tc.tile_pool(name="bbox", bufs=1) as bbp,
            tc.tile_pool(name="hub", bufs=1) as hubp,
            tc.tile_pool(name="expj", bufs=2) as expp,
            tc.tile_pool(name="lblchunk", bufs=4) as lblp,
            tc.tile_pool(name="small", bufs=2) as smallp,
            tc.tile_pool(name="mine", bufs=2) as minep,
            tc.tile_pool(name="psum", bufs=2, space="PSUM") as psump,
        ):
            def ap_band(dram, inner, g0, g1, p0, p1):
                return bass.AP(dram, p0 * G * inner + g0 * inner,
                               [[G * inner, p1 - p0], [1, (g1 - g0) * inner]])

            def dma_split(dst3, dram, inner, g0, g1, dstg0=None, full=None):
                """dst3[:, dstg0:dstg0+(g1-g0), :] <- dram groups [g0, g1).
                full=engine: one 128-partition transfer on that queue;
                else split top half (sync) / bottom half (gpsimd)."""
                if dstg0 is None:
                    dstg0 = g0
                ng = g1 - g0
                if full is not None:
                    full.dma_start(dst3[:, dstg0:dstg0 + ng, :],
                                   ap_band(dram, inner, g0, g1, 0, 128))
                    return
                nc.sync.dma_start(dst3[0:64, dstg0:dstg0 + ng, :],
                                  ap_band(dram, inner, g0, g1, 0, 64))
                nc.gpsimd.dma_start(dst3[64:128, dstg0:dstg0 + ng, :],
                                    ap_band(dram, inner, g0, g1, 64, 128))

            # ---- bulk DMA issue. Three queues: sync HWDGE + gpsimd SWDGE
            # stream top/bottom halves; the scalar/ACT HWDGE queue carries
            # every third pred sub-chunk and one sub-chunk of each of the
            # first 4 label chunks as full-width transfers, dispatched at
            # the head of the ACT instruction stream (all wait-free, so
            # they never block the exp/mining work behind them).
            abt = bbp.tile([128, G, 4], F32, tag="abt")
            pbt = bbp.tile([128, G, 4], F32, tag="pbt")

            pred = resp.tile([128, G, C], F32, tag="pred")
            lbls = [lblp.tile([128, CH, C], F32, tag="lbl", name=f"lbl{k}")
                    for k in range(NCH)]
            SUBL = CH // CHD    # label sub-chunks per chunk

            # scalar-queue dispatches first (ACT stream head, zero waits)
            for j in range(G // CHD):
                if j % 3 == 0:
                    dma_split(pred, d_pl, C, j * CHD, (j + 1) * CHD,
                              full=nc.scalar)
            for k in range(min(4, NCH)):
                dma_split(lbls[k], d_al, C, k * CH, k * CH + CHD,
                          dstg0=0, full=nc.scalar)

            # sync/gpsimd: bbox, then pred, then labels
            dma_split(abt, d_ab, 4, 0, G)
            dma_split(pbt, d_pb, 4, 0, G)
            for j in range(G // CHD):
                if j % 3 != 0:
                    dma_split(pred, d_pl, C, j * CHD, (j + 1) * CHD)

            # ---- constants (gpsimd stream: emitted before the label DMA
            # generation so nothing downstream waits on late Q7 work) ----
            blockones = constp.tile([128, 128], F32)
            nc.gpsimd.memset(blockones[:, :], 0.0)
            for r in range(BL):
                nc.gpsimd.memset(
                    blockones[r * PPR:(r + 1) * PPR, r * PPR:(r + 1) * PPR], 1.0)
            onescol = constp.tile([128, 1], F32)
            nc.gpsimd.memset(onescol[:, :], 1.0)
            fpart = constp.tile([128, NF], F32)
            nc.gpsimd.memset(fpart[:, :], 0.0)
            negone = constp.tile([128, 1], F32)
            nc.gpsimd.memset(negone[:, :], -1.0)
            negt0 = minep.tile([128, 1], F32, tag="negt")
            nc.gpsimd.memset(negt0[:, :], -T0)

            # ---- label chunk DMAs (pool bufs=4; dispatch waits resolve
            # instantly because the dots always run ahead) ----
            for k in range(NCH):
                j0 = 1 if k < 4 else 0   # sub-chunk 0 of chunks 0-3 on scalar
                for j in range(j0, SUBL):
                    dma_split(lbls[k], d_al, C, k * CH + j * CHD,
                              k * CH + (j + 1) * CHD, dstg0=j * CHD)

            # ---- bbox compute: posmask (pads read as positives), counts,
            # Huber loc sum ----
            absmax = bbp.tile([128, G], F32, tag="absmax")
            nc.vector.tensor_reduce(absmax[:, :], abt[:, :, :], AX.X, OP.max,
                                    apply_absolute_value=True)
            posmask = bbp.tile([128, G], F32, tag="posmask")
            nc.vector.tensor_scalar(posmask[:, :], absmax[:, :], 0.0, None, OP.is_gt)

            pospart = bbp.tile([128, 1], F32, tag="pospart")
            nc.vector.tensor_reduce(pospart[:, :], posmask[:, :], AX.X, OP.add)
            nc.vector.tensor_copy(fpart[:, COL_POS:COL_POS + 1], pospart[:, :])
            pos_rep = psump.tile([128, 1], F32, tag="posrep")
            nc.tensor.matmul(pos_rep[:, :], blockones[:, :], pospart[:, :])
            # posrep counts the 4 pads per row: k = 3*(posrep-4), and the
            # sign-count threshold negk2 = n_lanes + 24 - 6*posrep
            kcol = bbp.tile([128, 1], F32, tag="kcol")
            nc.vector.tensor_scalar(kcol[:, :], pos_rep[:, :], 3.0, -12.0,
                                    OP.mult, OP.add)
            negk2 = bbp.tile([128, 1], F32, tag="negk2")
            nc.vector.tensor_scalar(negk2[:, :], pos_rep[:, :], -6.0,
                                    float(PPR * G + 6 * NPAD), OP.mult, OP.add)

            # Huber loc loss (pads have pb==ab -> zero contribution)
            dt_ = hubp.tile([128, G, 4], F32, tag="hd")
            nc.vector.tensor_sub(dt_[:, :, :], pbt[:, :, :], abt[:, :, :])
            nc.scalar.activation(dt_[:, :, :], dt_[:, :, :], AF.Abs)  # a = |d|
            mt = hubp.tile([128, G, 4], F32, tag="hm")
            nc.vector.tensor_single_scalar(mt[:, :, :], dt_[:, :, :], 1.0, OP.min)
            st = hubp.tile([128, G, 4], F32, tag="hs")
            nc.scalar.activation(st[:, :, :], mt[:, :, :], AF.Square,
                                 scale=float(np.sqrt(0.5)))       # 0.5*m^2
            nc.scalar.activation(dt_[:, :, :], dt_[:, :, :], AF.Relu,
                                 bias=negone[:, :])               # relu(a-1)
            nc.vector.tensor_add(st[:, :, :], st[:, :, :], dt_[:, :, :])
            hpb = hubp.tile([128, G], F32, tag="hpb")
            nc.vector.tensor_reduce(hpb[:, :], st[:, :, :], AX.X, OP.add)
            hjunk = hubp.tile([128, G], F32, tag="hjunk")
            nc.vector.scalar_tensor_tensor(
                hjunk[:, :], hpb[:, :], 0.25, posmask[:, :], OP.mult, OP.mult,
                accum_out=fpart[:, COL_LOC:COL_LOC + 1])

            # ---- per-chunk exp + sumexp as pred chunks arrive ----
            sumexp = resp.tile([128, G], F32, tag="sumexp")
            for k in range(NCH):
                sl = pred[:, k * CH:(k + 1) * CH, :]
                ex = expp.tile([128, CH, C], F32, tag="exp")
                nc.scalar.activation(ex[:, :, :], sl, AF.Exp)
                nc.vector.tensor_reduce(sumexp[:, k * CH:(k + 1) * CH],
                                        ex[:, :, :], AX.X, OP.add)

            lse = resp.tile([128, G], F32, tag="lse")
            nc.scalar.activation(lse[:, :], sumexp[:, :], AF.Ln)
            pred0 = pred[:, :, 0]
            nconf = resp.tile([128, G], F32, tag="nconf")
            nc.vector.tensor_sub(nconf[:, :], lse[:, :], pred0)
            masked = resp.tile([128, G], F32, tag="masked")
            nc.vector.scalar_tensor_tensor(
                masked[:, :], posmask[:, :], NEG_BIG, nconf[:, :], OP.mult, OP.add)

            # S2, S3, S4 (host corrects S2 by the pads' 16*ln81)
            j2 = smallp.tile([128, G], F32, tag="sjunk")
            nc.vector.scalar_tensor_tensor(
                j2[:, :], posmask[:, :], 0.0, lse[:, :], OP.bypass, OP.mult,
                accum_out=fpart[:, COL_S2:COL_S2 + 1])
            j3 = smallp.tile([128, G], F32, tag="sjunk")
            nc.vector.scalar_tensor_tensor(
                j3[:, :], posmask[:, :], 0.0, pred0, OP.bypass, OP.mult,
                accum_out=fpart[:, COL_S3:COL_S3 + 1])
            nc.vector.tensor_reduce(fpart[:, COL_S4:COL_S4 + 1], pred0, AX.X, OP.add)

            # ---- hard-negative mining: binary search on t per row (ACT+PE;
            # runs while the label chunks stream) ----
            negt = negt0
            for i in range(NITER):
                cjunk = minep.tile([128, G], F32, tag="cjunk")
                cnt = minep.tile([128, 1], F32, tag="cnt")
                # sum(sign(masked - t)) = cnt_gt - cnt_le   (per partition)
                nc.scalar.activation(cjunk[:, :], masked[:, :], AF.Sign,
                                     bias=negt[:, :], accum_out=cnt[:, :])
                srep = psump.tile([128, 1], F32, tag="srep")
                nc.tensor.matmul(srep[:, :], blockones[:, :], cnt[:, :])
                # s = sign(sum_rep - (2k - n)) : +1 -> count>k -> t too low
                sdir = minep.tile([128, 1], F32, tag="sdir")
                nc.scalar.activation(sdir[:, :], srep[:, :], AF.Sign,
                                     bias=negk2[:, :])
                delta = T0 / (2 ** (i + 1))
                negt2 = minep.tile([128, 1], F32, tag="negt")
                nc.scalar.activation(negt2[:, :], sdir[:, :], AF.Identity,
                                     bias=negt[:, :], scale=-delta)
                negt = negt2

            # ---- label dot products (DVE), interleaved with the final
            # mining ops so nothing sits serially at the tail ----
            def emit_dot(k):
                # junk product tile: recycle the exp pool buffers (same
                # shape; exp work is fully done before the label phase)
                dj = expp.tile([128, CH, C], F32, tag="exp")
                nc.vector.scalar_tensor_tensor(
                    dj[:, :, :], lbls[k][:, :, :], 0.0,
                    pred[:, k * CH:(k + 1) * CH, :], OP.bypass, OP.mult,
                    accum_out=fpart[:, COL_S1 + k:COL_S1 + k + 1])

            for k in range(NCH):
                emit_dot(k)

            # final mining pass (needs negt, ready early in the label phase)
            tcol = minep.tile([128, 1], F32, tag="tcol")
            nc.vector.tensor_scalar(tcol[:, :], negt[:, :], -1.0, None, OP.mult)
            fjunk = minep.tile([128, G], F32, tag="fjunk")
            cntf = minep.tile([128, 1], F32, tag="cntf")
            nc.vector.tensor_scalar(fjunk[:, :], masked[:, :], tcol[:, :], 0.0,
                                    OP.is_gt, OP.add, accum_out=cntf[:, :])
            fjunk2 = minep.tile([128, G], F32, tag="fjunk")
            negsump = minep.tile([128, 1], F32, tag="negsump")
            nc.vector.scalar_tensor_tensor(
                fjunk2[:, :], masked[:, :], tcol[:, :], masked[:, :],
                OP.is_gt, OP.mult, accum_out=negsump[:, :])
            # contrib = negsum - t*cntf + t*kcol/PPR
            c1 = minep.tile([128, 1], F32, tag="c1")
            nc.vector.tensor_mul(c1[:, :], tcol[:, :], cntf[:, :])
            d1 = minep.tile([128, 1], F32, tag="d1")
            nc.vector.scalar_tensor_tensor(
                d1[:, :], kcol[:, :], 1.0 / PPR, tcol[:, :], OP.mult, OP.mult)
            e1 = minep.tile([128, 1], F32, tag="e1")
            nc.vector.tensor_sub(e1[:, :], negsump[:, :], c1[:, :])
            nc.vector.tensor_add(fpart[:, COL_NEG:COL_NEG + 1], e1[:, :], d1[:, :])

            # ---- final cross-partition reduce and output ----
            opsum = psump.tile([1, NF], F32, tag="opsum")
            nc.tensor.matmul(opsum[:, :], onescol[:, :], fpart[:, :])
            osb = constp.tile([1, NF], F32)
            nc.vector.tensor_copy(osb[:, :], opsum[:, :])
            nc.sync.dma_start(d_out[:, :], osb[:, :])

    nc.compile()
    return nc


def _prep(x, ncols, padval):
    """[B, P, ncols] -> [NCORES, 128, G*ncols] with padded anchors."""
    x = np.asarray(x, np.float32).reshape(B, P, ncols)
    pad = np.full((B, NPAD, ncols), padval, np.float32)
    xp = np.concatenate([x, pad], axis=1)          # [B, PPR*G, ncols]
    return xp.reshape(NCORES, BL * PPR, G * ncols)


def make_in_maps(actual_bbox_deltas, actual_labels, pred_bbox_deltas,
                 pred_labels):
    ab = _prep(actual_bbox_deltas, 4, 1.0)
    pb = _prep(pred_bbox_deltas, 4, 1.0)
    al = _prep(actual_labels, C, 0.0)
    pl = _prep(pred_labels, C, 0.0)
    return [{"ab": np.ascontiguousarray(ab[c]),
             "pb": np.ascontiguousarray(pb[c]),
             "al": np.ascontiguousarray(al[c]),
             "pl": np.ascontiguousarray(pl[c])} for c in range(NCORES)]


_nc = None


def combine(results):
    """results[core]["out"] -> (loc, conf) with host-side pad corrections."""
    ln81 = float(np.log(81.0))
    loc = conf = pos = 0.0
    for core in range(NCORES):
        o = results[core]["out"][0].astype(np.float64)
        s1 = o[COL_S1:COL_S1 + NCH].sum()
        s2 = o[COL_S2] - BL * NPAD * ln81
        loc += o[COL_LOC]
        conf += s2 - s1 + o[COL_S4] - o[COL_S3] + o[COL_NEG]
        pos += o[COL_POS] - BL * NPAD
    if pos == 0:
        return (np.float32(0.0), np.float32(0.0))
    return (np.float32(loc / pos), np.float32(conf / pos))


def kernel(actual_bbox_deltas, actual_labels, pred_bbox_deltas, pred_labels):
    global _nc
    if _nc is None:
        _nc = build()
    in_maps = make_in_maps(actual_bbox_deltas, actual_labels,
                           pred_bbox_deltas, pred_labels)
    res = run_bass_kernel_spmd(_nc, in_maps, core_ids=list(range(NCORES)))
    return combine(res.results)
